# revision 5
# baseline (speedup 1.0000x reference)
"""PixelShuffle1d (upscale=4) Trainium2 Bass kernel.

Full input x: (8, 256, 16384) f32 -> output (8, 64, 65536) f32 with
    out[b, c, 4*l + j] = x[b, 4*c + j, l]

Sharding: batch dim across 8 NeuronCores (1 batch row per core), no
communication. Per core this is pure data movement (16 MiB in + 16 MiB out),
HBM-bandwidth bound (~358 GB/s per NC -> ~94 us roofline).

Raw-bass SPMD program (per core), fully unrolled, manual semaphores:
partition p = 64*h + c', c' in [0,64) = output channel, h in {0,1} = L-half.
Per l-chunk t of F elements (ping-pong buffered, per-half chains):
  load_h  (SP HWDGE) : A[64h+c', s, j, l] = x[4c'+j, h*8192 + t*F + l]
  shuffle            : B[p, s, 4l+j] = A[p, s, j, l]
                       (h=0 on DVE, h=1 on ACT; strided 1x copy)
  store_h (ACT HWDGE): y[c', h*32768 + t*4F + m] = B[64h+c', s, m]
Halves use disjoint SBUF ports / SDMA engines (partitions 0-63 <-> even
ports, 64-127 <-> odd), so the per-half DMAs overlap; loads ride the SP
HWDGE ring, stores the ACT ring. DVE/ACT shuffle time (~8.6/7.0 us per
tile) hides under the ~23 us of DMA per tile.
"""

import numpy as np

import concourse.bass as bass
from concourse import mybir
from concourse.bass_utils import run_bass_kernel_spmd

B, C, L = 8, 256, 16384
U = 4
CO = C // U  # 64
LO = L * U  # 65536
NCORES = 8
HALF = L // 2  # 8192
F = 2048  # l-chunk per tile (per half)
NT = HALF // F  # 4 tiles
NSLOT = 2  # ping-pong


def build_nc() -> bass.Bass:
    nc = bass.Bass("TRN2", target_bir_lowering=False, debug=False)
    x = nc.dram_tensor("x", [C, L], mybir.dt.float32, kind="ExternalInput").ap()
    y = nc.dram_tensor("y", [CO, LO], mybir.dt.float32, kind="ExternalOutput").ap()

    xv = x.rearrange("(c j) (h l) -> h c j l", j=U, h=2)  # (2, 64, 4, 8192)
    yv = y.rearrange("c (h m) -> h c m", h=2)  # (2, 64, 32768)

    with (
        nc.sbuf_tensor("A", [128, NSLOT, U, F], mybir.dt.float32) as A,
        nc.sbuf_tensor("Bt", [128, NSLOT, U * F], mybir.dt.float32) as Bt,
        nc.semaphore("s_load0") as s_load0,
        nc.semaphore("s_load1") as s_load1,
        nc.semaphore("s_store0") as s_store0,
        nc.semaphore("s_store1") as s_store1,
        nc.semaphore("s_cp_dve") as s_cp_dve,
        nc.semaphore("s_cp_act") as s_cp_act,
        nc.Block() as block,
    ):
        s_load = (s_load0, s_load1)
        s_store = (s_store0, s_store1)
        Av = A.ap()  # (128, NSLOT, U, F)
        Bv = Bt.ap()  # (128, NSLOT, U*F)
        # b[p, s, 4l+j] view as (p, s, j, l) for the shuffle write
        Bi = Bv.rearrange("p s (l j) -> p s j l", j=U)

        @block.sync
        def _(sync: bass.BassEngine):
            for t in range(NT):
                s = t % NSLOT
                if t >= NSLOT:
                    # A slot reuse: shuffles of t-NSLOT must be done
                    sync.wait_ge(s_cp_dve, t - NSLOT + 1)
                    sync.wait_ge(s_cp_act, t - NSLOT + 1)
                for h in range(2):
                    pl, ph = 64 * h, 64 * (h + 1)
                    sync.dma_start(
                        out=Av[pl:ph, s], in_=xv[h, :, :, t * F : (t + 1) * F]
                    ).then_inc(s_load[h], 16)

        @block.vector
        def _(vector: bass.BassEngine):
            for t in range(NT):
                s = t % NSLOT
                vector.wait_ge(s_load0, 16 * (t + 1))
                if t >= NSLOT:
                    # B slot reuse: store of t-NSLOT must be done
                    vector.wait_ge(s_store0, 16 * (t - NSLOT + 1))
                vector.tensor_copy(out=Bi[0:64, s], in_=Av[0:64, s]).then_inc(
                    s_cp_dve, 1
                )

        @block.scalar
        def _(scalar: bass.BassEngine):
            for t in range(NT):
                s = t % NSLOT
                scalar.wait_ge(s_load1, 16 * (t + 1))
                if t >= NSLOT:
                    scalar.wait_ge(s_store1, 16 * (t - NSLOT + 1))
                scalar.copy(out=Bi[64:128, s], in_=Av[64:128, s]).then_inc(
                    s_cp_act, 1
                )
                # own copy retired (results committed) before store reads B
                scalar.wait_ge(s_cp_act, t + 1)
                scalar.dma_start(
                    out=yv[1, :, t * U * F : (t + 1) * U * F], in_=Bv[64:128, s]
                ).then_inc(s_store1, 16)
                scalar.wait_ge(s_cp_dve, t + 1)
                scalar.dma_start(
                    out=yv[0, :, t * U * F : (t + 1) * U * F], in_=Bv[0:64, s]
                ).then_inc(s_store0, 16)
            # drain: all stores complete before kernel end
            scalar.wait_ge(s_store0, 16 * NT)
            scalar.wait_ge(s_store1, 16 * NT)

    return nc


_NC = None


def kernel(x: np.ndarray) -> np.ndarray:
    global _NC
    if _NC is None:
        _NC = build_nc()
    x = np.asarray(x)
    in_maps = [{"x": np.ascontiguousarray(x[i])} for i in range(NCORES)]
    res = run_bass_kernel_spmd(_NC, in_maps, list(range(NCORES))).results
    return np.stack([res[i]["y"] for i in range(NCORES)], axis=0)


# revision 6
# speedup vs baseline: 26.7255x; 26.7255x over previous
"""PixelShuffle1d (upscale=4) Trainium2 Bass kernel.

Full input x: (8, 256, 16384) f32 -> output (8, 64, 65536) f32 with
    out[b, c, 4*l + j] = x[b, 4*c + j, l]

Sharding: batch dim across 8 NeuronCores (1 batch row per core), no
communication. Per core this is pure data movement (16 MiB in + 16 MiB out),
HBM-bandwidth bound (~358 GB/s per NC -> ~94 us roofline).

Raw-bass SPMD program (per core), fully unrolled, manual semaphores:
partition p = 64*h + c', c' in [0,64) = output channel, h in {0,1} = L-half.
Per l-chunk t of F elements (ping-pong buffered, per-half chains):
  load_h  (SP HWDGE) : A[64h+c', s, j, l] = x[4c'+j, h*8192 + t*F + l]
  shuffle            : B[p, s, 4l+j] = A[p, s, j, l]
                       (h=0 on DVE, h=1 on ACT; strided 1x copy)
  store_h (ACT HWDGE): y[c', h*32768 + t*4F + m] = B[64h+c', s, m]
Halves use disjoint SBUF ports / SDMA engines (partitions 0-63 <-> even
ports, 64-127 <-> odd), so the per-half DMAs overlap; loads ride the SP
HWDGE ring, stores the ACT ring. DVE/ACT shuffle time (~8.6/7.0 us per
tile) hides under the ~23 us of DMA per tile.
"""

import numpy as np

import concourse.bass as bass
from concourse import mybir
from concourse.bass_utils import run_bass_kernel_spmd

B, C, L = 8, 256, 16384
U = 4
CO = C // U  # 64
LO = L * U  # 65536
NCORES = 8
HALF = L // 2  # 8192
F = 2048  # l-chunk per tile (per half)
NT = HALF // F  # 4 tiles
NSLOT = 2  # ping-pong


def build_nc(repeats: int = 1) -> bass.Bass:
    """repeats > 1 re-runs the whole pass over the data in one NEFF launch
    (same output); used only for slope-based device timing in bench.py."""
    NI = NT * repeats

    nc = bass.Bass("TRN2", target_bir_lowering=False, debug=False)
    x = nc.dram_tensor("x", [C, L], mybir.dt.float32, kind="ExternalInput").ap()
    y = nc.dram_tensor("y", [CO, LO], mybir.dt.float32, kind="ExternalOutput").ap()

    xv = x.rearrange("(c j) (h l) -> h c j l", j=U, h=2)  # (2, 64, 4, 8192)
    yv = y.rearrange("c (h m) -> h c m", h=2)  # (2, 64, 32768)

    with (
        nc.sbuf_tensor("A", [128, NSLOT, U, F], mybir.dt.float32) as A,
        nc.sbuf_tensor("Bt", [128, NSLOT, U * F], mybir.dt.float32) as Bt,
        nc.semaphore("s_load0") as s_load0,
        nc.semaphore("s_load1") as s_load1,
        nc.semaphore("s_store0") as s_store0,
        nc.semaphore("s_store1") as s_store1,
        nc.semaphore("s_cp_dve") as s_cp_dve,
        nc.semaphore("s_cp_act") as s_cp_act,
        nc.Block() as block,
    ):
        s_load = (s_load0, s_load1)
        Av = A.ap()  # (128, NSLOT, U, F)
        Bv = Bt.ap()  # (128, NSLOT, U*F)
        # b[p, s, 4l+j] view as (p, s, j, l) for the shuffle write
        Bi = Bv.rearrange("p s (l j) -> p s j l", j=U)

        @block.sync
        def _(sync: bass.BassEngine):
            for t in range(NI):
                tt = t % NT
                s = t % NSLOT
                if t >= NSLOT:
                    # A slot reuse: shuffles of t-NSLOT must be done
                    sync.wait_ge(s_cp_dve, t - NSLOT + 1)
                    sync.wait_ge(s_cp_act, t - NSLOT + 1)
                for h in range(2):
                    pl, ph = 64 * h, 64 * (h + 1)
                    sync.dma_start(
                        out=Av[pl:ph, s], in_=xv[h, :, :, tt * F : (tt + 1) * F]
                    ).then_inc(s_load[h], 16)

        @block.vector
        def _(vector: bass.BassEngine):
            for t in range(NI):
                s = t % NSLOT
                vector.wait_ge(s_load0, 16 * (t + 1))
                if t >= NSLOT:
                    # B slot reuse: store of t-NSLOT must be done
                    vector.wait_ge(s_store0, 16 * (t - NSLOT + 1))
                vector.tensor_copy(out=Bi[0:64, s], in_=Av[0:64, s]).then_inc(
                    s_cp_dve, 1
                )

        @block.scalar
        def _(scalar: bass.BassEngine):
            for t in range(NI):
                tt = t % NT
                s = t % NSLOT
                scalar.wait_ge(s_load1, 16 * (t + 1))
                if t >= NSLOT:
                    scalar.wait_ge(s_store1, 16 * (t - NSLOT + 1))
                scalar.copy(out=Bi[64:128, s], in_=Av[64:128, s]).then_inc(
                    s_cp_act, 1
                )
                # own copy retired (results committed) before store reads B
                scalar.wait_ge(s_cp_act, t + 1)
                scalar.dma_start(
                    out=yv[1, :, tt * U * F : (tt + 1) * U * F], in_=Bv[64:128, s]
                ).then_inc(s_store1, 16)
                scalar.wait_ge(s_cp_dve, t + 1)
                scalar.dma_start(
                    out=yv[0, :, tt * U * F : (tt + 1) * U * F], in_=Bv[0:64, s]
                ).then_inc(s_store0, 16)
            # drain: all stores complete before kernel end
            scalar.wait_ge(s_store0, 16 * NI)
            scalar.wait_ge(s_store1, 16 * NI)

    return nc


_NC = None


def kernel(x: np.ndarray) -> np.ndarray:
    global _NC
    if _NC is None:
        _NC = build_nc()
    x = np.asarray(x)
    in_maps = [{"x": np.ascontiguousarray(x[i])} for i in range(NCORES)]
    res = run_bass_kernel_spmd(_NC, in_maps, list(range(NCORES))).results
    return np.stack([res[i]["y"] for i in range(NCORES)], axis=0)


# revision 11
# speedup vs baseline: 31.6341x; 1.1837x over previous
"""PixelShuffle1d (upscale=4) Trainium2 Bass kernel.

Full input x: (8, 256, 16384) f32 -> output (8, 64, 65536) f32 with
    out[b, c, 4*l + j] = x[b, 4*c + j, l]

Sharding: batch dim across 8 NeuronCores (1 batch row per core), no
communication. Per core this is pure data movement (16 MiB in + 16 MiB out),
HBM-bandwidth bound (~358 GB/s per NC -> ~94 us roofline).

Raw-bass SPMD program (per core), fully unrolled, manual semaphores:
partition p = 64*h + c', c' in [0,64) = output channel, h in {0,1} = L-half.
Per l-chunk t of F elements (ping-pong buffered, per-half chains):
  load_h  (SP HWDGE) : A[64h+c', s, j, l] = x[4c'+j, h*8192 + t*F + l]
  shuffle            : B[p, s, 4l+j] = A[p, s, j, l]
                       (h=0 on DVE, h=1 on ACT; strided 1x copy)
  store_h (ACT HWDGE): y[c', h*32768 + t*4F + m] = B[64h+c', s, m]
Halves use disjoint SBUF ports / SDMA engines (partitions 0-63 <-> even
ports, 64-127 <-> odd), so the per-half DMAs overlap; loads ride the SP
HWDGE ring, stores the ACT ring. DVE/ACT shuffle time (~8.6/7.0 us per
tile) hides under the ~23 us of DMA per tile.
"""

import numpy as np

import concourse.bass as bass
from concourse import mybir
from concourse.bass_utils import run_bass_kernel_spmd

B, C, L = 8, 256, 16384
U = 4
CO = C // U  # 64
LO = L * U  # 65536
NCORES = 8
HALF = L // 2  # 8192
F = 2048  # l-chunk per tile (per half)
NT = HALF // F  # 4 tiles
NSA = 3  # A-tile slots (load run-ahead depth)
NSB = 2  # B-tile slots


def build_nc(repeats: int = 1) -> bass.Bass:
    """repeats > 1 re-runs the whole pass over the data in one NEFF launch
    (same output); used only for slope-based device timing in bench.py."""
    NI = NT * repeats

    nc = bass.Bass("TRN2", target_bir_lowering=False, debug=False)
    x = nc.dram_tensor("x", [C, L], mybir.dt.float32, kind="ExternalInput").ap()
    y = nc.dram_tensor("y", [CO, LO], mybir.dt.float32, kind="ExternalOutput").ap()

    xv = x.rearrange("(c j) (h l) -> h c j l", j=U, h=2)  # (2, 64, 4, 8192)
    yv = y.rearrange("c (h m) -> h c m", h=2)  # (2, 64, 32768)

    with (
        nc.sbuf_tensor("A", [128, NSA, U, F], mybir.dt.float32) as A,
        nc.sbuf_tensor("Bt", [128, NSB, U * F], mybir.dt.float32) as Bt,
        nc.semaphore("s_load0") as s_load0,
        nc.semaphore("s_load1") as s_load1,
        nc.semaphore("s_store0") as s_store0,
        nc.semaphore("s_store1") as s_store1,
        nc.semaphore("s_cp_dve") as s_cp_dve,
        nc.semaphore("s_cp_act") as s_cp_act,
        nc.Block() as block,
    ):
        s_load = (s_load0, s_load1)
        Av = A.ap()  # (128, NSLOT, U, F)
        Bv = Bt.ap()  # (128, NSLOT, U*F)
        # b[p, s, 4l+j] view as (p, s, j, l) for the shuffle write
        Bi = Bv.rearrange("p s (l j) -> p s j l", j=U)

        def dispatch_load(eng, h, t, sem):
            tt = t % NT
            s = t % NSA
            pl, ph = 64 * h, 64 * (h + 1)
            eng.dma_start(
                out=Av[pl:ph, s], in_=xv[h, :, :, tt * F : (tt + 1) * F]
            ).then_inc(sem, 16)

        @block.sync
        def _(sync: bass.BassEngine):
            # h0 loads ride the SP HWDGE ring
            for t in range(NI):
                if t >= NSA:
                    # A[0:64] slot reuse: DVE shuffle of t-NSA must be done
                    sync.wait_ge(s_cp_dve, t - NSA + 1)
                dispatch_load(sync, 0, t, s_load0)

        @block.vector
        def _(vector: bass.BassEngine):
            for t in range(NI):
                s = t % NSB
                vector.wait_ge(s_load0, 16 * (t + 1))
                if t >= NSB:
                    # B slot reuse: store of t-NSB must be done
                    vector.wait_ge(s_store0, 16 * (t - NSB + 1))
                vector.tensor_copy(
                    out=Bi[0:64, s], in_=Av[0:64, t % NSA]
                ).then_inc(s_cp_dve, 1)

        @block.scalar
        def _(scalar: bass.BassEngine):
            # h1 loads ride the ACT HWDGE ring; h1 shuffles run on ACT.
            # Loads are dispatched NSA-1 iterations ahead of the shuffles so
            # the ring stays fed while ACT computes.
            for t in range(NSA - 1):
                dispatch_load(scalar, 1, t, s_load1)
            for t in range(NI):
                s = t % NSB
                if t + NSA - 1 < NI:
                    # dispatch load t+NSA-1 into the slot our shuffle t-1
                    # read; wait for that shuffle to retire first
                    if t >= 1:
                        scalar.wait_ge(s_cp_act, t)
                    dispatch_load(scalar, 1, t + NSA - 1, s_load1)
                scalar.wait_ge(s_load1, 16 * (t + 1))
                if t >= NSB:
                    scalar.wait_ge(s_store1, 16 * (t - NSB + 1))
                scalar.copy(
                    out=Bi[64:128, s], in_=Av[64:128, t % NSA]
                ).then_inc(s_cp_act, 1)

        @block.gpsimd
        def _(g: bass.BassEngine):
            # stores ride the SWDGE ring so load/store queues never couple
            for t in range(NI):
                tt = t % NT
                s = t % NSB
                g.wait_ge(s_cp_act, t + 1)
                g.dma_start(
                    out=yv[1, :, tt * U * F : (tt + 1) * U * F], in_=Bv[64:128, s]
                ).then_inc(s_store1, 16)
                g.wait_ge(s_cp_dve, t + 1)
                g.dma_start(
                    out=yv[0, :, tt * U * F : (tt + 1) * U * F], in_=Bv[0:64, s]
                ).then_inc(s_store0, 16)
            # drain: all stores complete before kernel end
            g.wait_ge(s_store0, 16 * NI)
            g.wait_ge(s_store1, 16 * NI)

    return nc


_NC = None


def kernel(x: np.ndarray) -> np.ndarray:
    global _NC
    if _NC is None:
        _NC = build_nc()
    x = np.asarray(x)
    in_maps = [{"x": np.ascontiguousarray(x[i])} for i in range(NCORES)]
    res = run_bass_kernel_spmd(_NC, in_maps, list(range(NCORES))).results
    return np.stack([res[i]["y"] for i in range(NCORES)], axis=0)
